# revision 1
# baseline (speedup 1.0000x reference)
"""Trainium2 Bass kernel for nn_BoundaryAttentionHead (gnn_message_passing).

reference computation:
    diff = (x[src] - x[dst])**2                    # [E, C]
    variance = scatter_add(diff, dst) / k          # [N, C]
    h = relu(variance @ W1 + b1)                   # [N, 64]
    out = sigmoid(h @ W2 + b2)                     # [N, 1]

Sharding: nodes across 8 cores (dst-segment partition); MLP weights
replicated; W1 pre-scaled by 1/k on host.

Per-node expansion (avoids per-edge subtraction):
    sum_j (x_sj - x_n)^2 = S2 - 2 x_n . S1 + deg * x_n^2
    S1 = sum_j x_sj,  S2 = sum_j x_sj^2

"gather" mode: batched SWDGE dma_gather (int16 indices) over four
32768-row windows of a zero-row-augmented copy of x, spread over 4 SWDGE
queues. Per 128-node tile and window class q: A aligned columns
(node-major; holes -> zero row) + overflow columns (dense packing of the
>A leftovers) routed back to their node with a one-hot (Sel) matmul on PE.

"indirect" mode (BAH_MODE=indirect): simple fallback, one indirect DMA per
(tile, slot); ~4.7x slower but trivially correct.
"""
import os
import sys
import types

import numpy as np

_KERNEL_CACHE = {}
MODE = os.environ.get("BAH_MODE", "gather")

P = 128
NCLASS = 4
CHUNK = 25000
WIN = 32768
GT = 5  # tiles per group
ALIGN_A = 2


def _install_ntff_hook():
    if "antenv.axon_hooks" in sys.modules:
        return
    sys.path.insert(0, "/root/.axon_site")
    try:
        from trn_agent_boot.trn_boot import _ntff_profile_via_ctypes
    except Exception:
        return
    mod = types.ModuleType("antenv.axon_hooks")
    _hook = [_ntff_profile_via_ctypes("/opt/axon/libaxon_pjrt.so")]
    mod.get_axon_ntff_profile_hook = lambda: _hook[0]
    mod.set_axon_ntff_profile_hook = lambda h: _hook.__setitem__(0, h)
    sys.modules["antenv.axon_hooks"] = mod


# ---------------------------------------------------------------- host side


def _node_lists(x, src, dst, k):
    """Per-node neighbour lists (-1 padded), degrees, per-core node ranges."""
    N, C = x.shape
    E = src.shape[0]
    n_cores = 8
    base = N // n_cores
    rem = N % n_cores
    starts = [c * base + min(c, rem) for c in range(n_cores)] + [N]

    src = np.asarray(src)
    dst = np.asarray(dst)

    fast = False
    if E % N == 0 and E // N > 0:
        K = E // N
        fast = np.array_equal(dst, np.repeat(np.arange(N, dtype=dst.dtype), K))

    if fast:
        nbrs = src.reshape(N, E // N).astype(np.int64)
        deg = np.full(N, E // N, dtype=np.int64)
    else:
        order = np.argsort(dst, kind="stable")
        ds = dst[order].astype(np.int64)
        ss = src[order].astype(np.int64)
        deg = np.bincount(ds, minlength=N)
        Kmax = int(deg.max()) if E else 1
        nbrs = np.full((N, Kmax), -1, dtype=np.int64)
        seg_off = np.zeros(N + 1, dtype=np.int64)
        np.cumsum(deg, out=seg_off[1:])
        pos = np.arange(E, dtype=np.int64) - seg_off[ds]
        nbrs[ds, pos] = ss
    return starts, nbrs, deg


def _build_x_aug(x):
    """Four [WIN, C] tables: row 0 zero, rows 1.. = x[CHUNK*q : CHUNK*q+WIN-1]."""
    N, C = x.shape
    tabs = []
    for q in range(NCLASS):
        t = np.zeros((WIN, C), dtype=np.float32)
        lo = CHUNK * q
        hi = min(N, lo + WIN - 1)
        if hi > lo:
            t[1 : 1 + hi - lo] = np.asarray(x[lo:hi], dtype=np.float32)
        tabs.append(t)
    return tabs


def _plan_core(nbrs_core, A, NT):
    """Plan one core. Returns (aligned [NT,4,A,128] i16,
    ov: {(t,q): (refs i16 [m*128], tgts i32 [m*128])}, ov_needed [NT,4])."""
    aligned = np.zeros((NT, NCLASS, A, P), dtype=np.int16)
    ov = {}
    ov_needed = np.zeros((NT, NCLASS), dtype=np.int64)
    n = nbrs_core.shape[0]
    for t in range(NT):
        blk = nbrs_core[t * P : min(n, (t + 1) * P)]  # [npn, K]
        npn = blk.shape[0]
        for q in range(NCLASS):
            sel = (blk >= CHUNK * q) & (blk < CHUNK * (q + 1))
            rank = np.cumsum(sel, axis=1) - 1  # within-row rank where sel
            refs = (blk - CHUNK * q + 1).astype(np.int32)
            # aligned part
            am = sel & (rank < A)
            pp, jj = np.nonzero(am)
            aligned[t, q, rank[pp, jj], pp] = refs[pp, jj].astype(np.int16)
            # overflow part (row-major nonzero => grouped by p)
            om = sel & (rank >= A)
            po, jo = np.nonzero(om)
            cnt = len(po)
            m = (cnt + P - 1) // P
            ov_needed[t, q] = m
            if m:
                orf = np.zeros(m * P, dtype=np.int16)
                otg = np.full(m * P, 200, dtype=np.int32)
                orf[:cnt] = refs[po, jo].astype(np.int16)
                otg[:cnt] = po
                ov[(t, q)] = (orf, otg)
    return aligned, ov, ov_needed


def _wrap_call(vals):
    """Pack one column's 128 int16 values into its [128, 8] SBUF idx block:
    call-flat element i -> (i%16, i//16), replicated x8 down partitions."""
    a = np.zeros((16, 8), dtype=np.int16)
    i = np.arange(P)
    a[i % 16, i // 16] = vals
    return np.tile(a, (8, 1))


def _layout_groups(NT, OVC, A):
    """Class blocks are padded to a uniform per-group width so a tile's
    aligned columns across all 4 classes form a regular 4D access pattern
    (enables a single DVE reduce per tile)."""
    groups = []
    for g0 in range(0, NT, GT):
        tiles = list(range(g0, min(NT, g0 + GT)))
        ngt = len(tiles)
        col = 0
        qblocks = []
        acol = {}
        ocol = {}
        ovlist = []
        for q in range(NCLASS):
            qstart = col
            for t in tiles:
                acol[(t, q)] = col
                col += A
            for t in tiles:
                ocol[(t, q)] = col
                for cc in range(int(OVC[t, q])):
                    ovlist.append((t, q, cc))
                col += int(OVC[t, q])
            qblocks.append((qstart, col))
        groups.append(
            dict(tiles=tiles, ngt=ngt, ncol=col, qblocks=qblocks,
                 acol=acol, ocol=ocol, ovlist=ovlist)
        )
    return groups


def _plan_all(x, src, dst, k, A=ALIGN_A):
    N, C = x.shape
    starts, nbrs, deg = _node_lists(x, src, dst, k)
    n_cores = 8
    per_core = max(starts[c + 1] - starts[c] for c in range(n_cores))
    NT = (per_core + P - 1) // P
    n_nodes_pad = NT * P

    cores = []
    for c in range(n_cores):
        lo, hi = starts[c], starts[c + 1]
        nb = np.full((n_nodes_pad, nbrs.shape[1]), -1, dtype=np.int64)
        nb[: hi - lo] = nbrs[lo:hi]
        cores.append(_plan_core(nb, A, NT))

    OVC = np.zeros((NT, NCLASS), dtype=np.int64)
    for _, _, ovn in cores:
        OVC = np.maximum(OVC, ovn)

    groups = _layout_groups(NT, OVC, A)
    total_cols = sum(g["ncol"] for g in groups)
    total_ovcols = sum(len(g["ovlist"]) for g in groups)

    core_data = []
    for c in range(n_cores):
        al, ov, _ = cores[c]
        idx16 = np.zeros((P, total_cols * 8), dtype=np.int16)
        ovt = np.full((P, max(total_ovcols, 1)), 200.0, dtype=np.float32)
        colbase = 0
        ovbase = 0
        for g in groups:
            for q in range(NCLASS):
                for t in g["tiles"]:
                    a0 = g["acol"][(t, q)]
                    for a in range(A):
                        j = colbase + a0 + a
                        idx16[:, j * 8 : (j + 1) * 8] = _wrap_call(al[t, q, a])
            for li, (t, q, cc) in enumerate(g["ovlist"]):
                j = colbase + g["ocol"][(t, q)] + cc
                rr, tt = ov.get((t, q), (None, None))
                if rr is None:
                    vals = np.zeros(P, dtype=np.int16)
                    tgts = np.full(P, 200, dtype=np.int32)
                else:
                    if len(rr) < (cc + 1) * P:
                        rr = np.concatenate(
                            [rr, np.zeros((cc + 1) * P - len(rr), np.int16)]
                        )
                        tt = np.concatenate(
                            [tt, np.full((cc + 1) * P - len(tt), 200, np.int32)]
                        )
                    vals = rr[cc * P : (cc + 1) * P]
                    tgts = tt[cc * P : (cc + 1) * P]
                idx16[:, j * 8 : (j + 1) * 8] = _wrap_call(vals)
                ovt[:, ovbase + li] = tgts.astype(np.float32)
            colbase += g["ncol"]
            ovbase += len(g["ovlist"])
        core_data.append(dict(idx16=idx16, ovt=ovt))

    return dict(
        N=N, C=C, NT=NT, n_nodes_pad=n_nodes_pad, A=A,
        starts=starts, deg=deg, OVC=OVC, groups=groups,
        total_cols=total_cols, total_ovcols=total_ovcols,
        core_data=core_data,
    )


# ------------------------------------------------------------- device side


def _build_gather(plan, H):
    import concourse.bacc as bacc
    import concourse.tile as tile
    from concourse import mybir
    from concourse.library_config import mlp
    from concourse.masks import make_identity

    F32 = mybir.dt.float32
    BF16 = mybir.dt.bfloat16
    I16 = mybir.dt.int16
    C = plan["C"]
    A = plan["A"]
    groups = plan["groups"]
    n_nodes_pad = plan["n_nodes_pad"]

    nc = bacc.Bacc("TRN2", num_swdge_queues=4, dynamic_dma_scratch_size=32768)
    xq_t = [
        nc.dram_tensor(f"xq{q}", [WIN, C], F32, kind="ExternalInput")
        for q in range(NCLASS)
    ]
    idx16 = nc.dram_tensor(
        "idx16", [P, plan["total_cols"] * 8], I16, kind="ExternalInput"
    )
    ovt_d = nc.dram_tensor(
        "ovt", [P, max(plan["total_ovcols"], 1)], F32, kind="ExternalInput"
    )
    xloc = nc.dram_tensor("xloc", [n_nodes_pad, C], F32, kind="ExternalInput")
    sdeg = nc.dram_tensor("sdeg", [n_nodes_pad, 1], F32, kind="ExternalInput")
    iota = nc.dram_tensor("iota", [P, P], F32, kind="ExternalInput")
    w1k = nc.dram_tensor("w1k", [C, H], F32, kind="ExternalInput")
    b1 = nc.dram_tensor("b1", [H, 1], F32, kind="ExternalInput")
    w2 = nc.dram_tensor("w2", [H, 1], F32, kind="ExternalInput")
    b2 = nc.dram_tensor("b2", [1, 1], F32, kind="ExternalInput")
    y = nc.dram_tensor("y", [1, n_nodes_pad], F32, kind="ExternalOutput")

    qrr = [0]  # SWDGE queue round-robin

    with tile.TileContext(nc) as tc:
        with tc.tile_critical():
            nc.gpsimd.load_library(mlp)
        with (
            tc.tile_pool(name="const", bufs=1) as cpool,
            tc.tile_pool(name="grp", bufs=3) as gpool,
            tc.tile_pool(name="spool", bufs=2 * GT + 2) as spool,
            tc.tile_pool(name="selp", bufs=1) as selp,
            tc.tile_pool(name="sbuf", bufs=2) as pool,
            tc.tile_pool(name="hbuf", bufs=1) as hpool,
            tc.tile_pool(name="psum", bufs=2, space="PSUM") as psum,
            tc.tile_pool(name="opsum", bufs=1, space="PSUM") as opsum,
        ):
            ident = cpool.tile([P, P], F32)
            make_identity(nc, ident[:])
            iota_t = cpool.tile([P, P], F32)
            nc.sync.dma_start(out=iota_t[:], in_=iota[:])
            w1k_t = cpool.tile([C, H], F32)
            nc.sync.dma_start(out=w1k_t[:], in_=w1k[:])
            b1_t = cpool.tile([H, 1], F32)
            nc.sync.dma_start(out=b1_t[:], in_=b1[:])
            w2_t = cpool.tile([H, 1], F32)
            nc.sync.dma_start(out=w2_t[:], in_=w2[:])
            b2_t = cpool.tile([1, 1], F32)
            nc.sync.dma_start(out=b2_t[:], in_=b2[:])

            cum_col = [0]
            cum_ov = [0]
            for g in groups:
                cum_col.append(cum_col[-1] + g["ncol"])
                cum_ov.append(cum_ov[-1] + len(g["ovlist"]))

            ctxs = {}

            def emit_gathers(gi):
                g = groups[gi]
                ncol = g["ncol"]
                ovlist = g["ovlist"]
                novc = len(ovlist)
                ctx = {}
                grp = gpool.tile([P, ncol * C], F32, tag="grp")
                grpv = grp[:].rearrange("p (t c) -> p t c", c=C)
                idxg = pool.tile([P, ncol * 8], I16, tag="idxg")
                nc.sync.dma_start(
                    out=idxg[:],
                    in_=idx16[:, cum_col[gi] * 8 : (cum_col[gi] + ncol) * 8],
                )
                for q in range(NCLASS):
                    qs, qe = g["qblocks"][q]
                    c0 = qs
                    while c0 < qe:
                        run = min(8, qe - c0)
                        n = run * P
                        nc.gpsimd.dma_gather(
                            grpv[:, c0 : c0 + run, :],
                            xq_t[q][:],
                            idxg[:, c0 * 8 : (c0 + run) * 8],
                            n, n, C,
                            queue_num=qrr[0] % 4,
                        )
                        qrr[0] += 1
                        c0 += run
                ctx["grp"] = grp
                ctx["grpv"] = grpv
                if novc:
                    ovtg = pool.tile([P, novc], F32, tag="ovtg")
                    nc.sync.dma_start(
                        out=ovtg[:],
                        in_=ovt_d[:, cum_ov[gi] : cum_ov[gi] + novc],
                    )
                    sel_g = selp.tile([P, novc * P], BF16, tag="sel_g")
                    nc.vector.tensor_tensor(
                        out=sel_g[:].rearrange("p (o n) -> p o n", n=P),
                        in0=ovtg[:, :, None].to_broadcast([P, novc, P]),
                        in1=iota_t[:, None, :].to_broadcast([P, novc, P]),
                        op=mybir.AluOpType.is_equal,
                    )
                    ctx["sel_g"] = sel_g
                    ctx["ov_local"] = {
                        (t, q, cc): li for li, (t, q, cc) in enumerate(ovlist)
                    }
                ctxs[gi] = ctx

            def emit_compute(gi):
                g = groups[gi]
                ngt = g["ngt"]
                ovlist = g["ovlist"]
                novc = len(ovlist)
                ctx = ctxs.pop(gi)
                grp = ctx["grp"]
                grpv = ctx["grpv"]

                if novc:
                    sel_g = ctx["sel_g"]
                    ov_local = ctx["ov_local"]
                    ovbf = gpool.tile([P, novc * 2 * C], BF16, tag="ovbf")
                    ovbfv = ovbf[:].rearrange("p (t c) -> p t c", c=2 * C)
                    for q in range(NCLASS):
                        tq = [e for e in ovlist if e[1] == q]
                        if not tq:
                            continue
                        li0 = ov_local[tq[0]]
                        ov0 = g["ocol"][(tq[0][0], q)]
                        ovq = len(tq)
                        src_ap = grpv[:, ov0 : ov0 + ovq, :]
                        nc.vector.tensor_copy(
                            out=ovbfv[:, li0 : li0 + ovq, 0:C], in_=src_ap
                        )
                        nc.scalar.activation(
                            ovbfv[:, li0 : li0 + ovq, C : 2 * C],
                            src_ap,
                            mybir.ActivationFunctionType.Square,
                        )

                def _pairsum(t, tag):
                    blks = [
                        grp[:, g["acol"][(t, q)] * C : (g["acol"][(t, q)] + A) * C]
                        for q in range(NCLASS)
                    ]
                    t1 = pool.tile([P, A * C], F32, tag=tag + "a")
                    t2 = pool.tile([P, A * C], F32, tag=tag + "b")
                    nc.vector.tensor_add(out=t1[:], in0=blks[0], in1=blks[1])
                    nc.vector.tensor_add(out=t2[:], in0=blks[2], in1=blks[3])
                    nc.vector.tensor_add(out=t1[:], in0=t1[:], in1=t2[:])
                    s = spool.tile([P, C], F32, tag=tag)
                    nc.vector.tensor_add(
                        out=s[:], in0=t1[:, 0:C], in1=t1[:, C : A * C]
                    )
                    return s

                s1_list = {}
                s2_list = {}
                for t in g["tiles"]:
                    s1_list[t] = _pairsum(t, "s1")
                # squares in place: classes 0,1 on ACT; 2,3 on DVE
                for q in range(NCLASS):
                    a0 = g["acol"][(g["tiles"][0], q)]
                    ap = grp[:, a0 * C : (a0 + ngt * A) * C]
                    if q < 2:
                        nc.scalar.activation(
                            ap, ap, mybir.ActivationFunctionType.Square
                        )
                    else:
                        nc.vector.tensor_tensor(
                            out=ap, in0=ap, in1=ap, op=mybir.AluOpType.mult
                        )
                for t in g["tiles"]:
                    s2_list[t] = _pairsum(t, "s2")

                h_g = hpool.tile([H, GT * P], F32, tag="h_g")
                o_ps = opsum.tile([1, GT * P], F32, space="PSUM", tag="o_ps")
                y_sb = hpool.tile([1, GT * P], F32, tag="y_sb")

                for ti, t in enumerate(g["tiles"]):
                    s1 = s1_list[t]
                    s2 = s2_list[t]
                    mmlist = [
                        (q, cc)
                        for q in range(NCLASS)
                        for cc in range(int(plan["OVC"][t, q]))
                    ]
                    ovps = None
                    if mmlist:
                        ovps = psum.tile([P, 2 * C], F32, space="PSUM", tag="ovps")
                        for mi, (q, cc) in enumerate(mmlist):
                            li = ov_local[(t, q, cc)]
                            nc.tensor.matmul(
                                out=ovps[:],
                                lhsT=sel_g[:, li * P : (li + 1) * P],
                                rhs=ovbfv[:, li, :],
                                start=(mi == 0),
                                stop=(mi == len(mmlist) - 1),
                            )
                    xl = pool.tile([P, C], F32, tag="xl")
                    nc.sync.dma_start(out=xl[:], in_=xloc[t * P : (t + 1) * P, :])
                    sd = pool.tile([P, 1], F32, tag="sd")
                    nc.sync.dma_start(out=sd[:], in_=sdeg[t * P : (t + 1) * P, :])
                    if ovps is not None:
                        s1t = pool.tile([P, C], F32, tag="s1t")
                        s2t = pool.tile([P, C], F32, tag="s2t")
                        nc.vector.tensor_add(out=s1t[:], in0=s1[:], in1=ovps[:, 0:C])
                        nc.vector.tensor_add(
                            out=s2t[:], in0=s2[:], in1=ovps[:, C : 2 * C]
                        )
                    else:
                        s1t, s2t = s1, s2
                    # V = S2t + (-2 xl) . S1t + deg xl^2
                    xlm2 = pool.tile([P, C], F32, tag="xlm2")
                    nc.scalar.mul(out=xlm2[:], in_=xl[:], mul=-2.0)
                    m_t = pool.tile([P, C], F32, tag="m_t")
                    nc.vector.tensor_tensor(
                        out=m_t[:], in0=xlm2[:], in1=s1t[:],
                        op=mybir.AluOpType.mult,
                    )
                    u_t = pool.tile([P, C], F32, tag="u_t")
                    nc.scalar.activation(
                        u_t[:], xl[:], mybir.ActivationFunctionType.Square,
                        scale=sd[:, :1],
                    )
                    v_t = pool.tile([P, C], F32, tag="v_t")
                    nc.vector.tensor_add(out=v_t[:], in0=s2t[:], in1=m_t[:])
                    nc.vector.tensor_add(out=v_t[:], in0=v_t[:], in1=u_t[:])
                    vt_ps = psum.tile([C, P], F32, space="PSUM", tag="vt")
                    nc.tensor.transpose(out=vt_ps[:], in_=v_t[:], identity=ident[:])
                    vt = pool.tile([C, P], F32, tag="vts")
                    nc.scalar.copy(out=vt[:], in_=vt_ps[:])
                    h_ps = psum.tile([H, P], F32, space="PSUM", tag="h_ps")
                    nc.tensor.matmul(
                        out=h_ps[:], lhsT=w1k_t[:], rhs=vt[:], start=True, stop=True
                    )
                    nc.scalar.activation(
                        h_g[:, ti * P : (ti + 1) * P],
                        h_ps[:],
                        mybir.ActivationFunctionType.Relu,
                        bias=b1_t[:, :1],
                    )

                for s in range(0, ngt * P, 512):
                    e = min(s + 512, ngt * P)
                    nc.tensor.matmul(
                        out=o_ps[:, s:e], lhsT=w2_t[:], rhs=h_g[:, s:e],
                        start=True, stop=True,
                    )
                nc.scalar.activation(
                    y_sb[:, : ngt * P],
                    o_ps[:, : ngt * P],
                    mybir.ActivationFunctionType.Sigmoid,
                    bias=b2_t[:, :1],
                )
                g0 = g["tiles"][0]
                nc.sync.dma_start(
                    out=y[:, g0 * P : g0 * P + ngt * P], in_=y_sb[:, : ngt * P]
                )

            for gi in range(len(groups) + 1):
                if gi < len(groups):
                    emit_gathers(gi)
                if gi >= 1:
                    emit_compute(gi - 1)
    nc.compile()
    return nc


# ------------------------------------------------------- v1 fallback build


def _build_indirect(N, C, KS, NT, n_nodes_pad, H):
    import concourse.bass as bass
    import concourse.bacc as bacc
    import concourse.tile as tile
    from concourse import mybir
    from concourse.masks import make_identity

    F32 = mybir.dt.float32
    I32 = mybir.dt.int32
    OG = 8

    nc = bacc.Bacc("TRN2")
    x = nc.dram_tensor("x", [N, C], F32, kind="ExternalInput")
    idx = nc.dram_tensor("idx", [n_nodes_pad, KS], I32, kind="ExternalInput")
    w1k = nc.dram_tensor("w1k", [C, H], F32, kind="ExternalInput")
    b1 = nc.dram_tensor("b1", [H, 1], F32, kind="ExternalInput")
    w2 = nc.dram_tensor("w2", [H, 1], F32, kind="ExternalInput")
    b2 = nc.dram_tensor("b2", [1, 1], F32, kind="ExternalInput")
    y = nc.dram_tensor("y", [1, n_nodes_pad], F32, kind="ExternalOutput")

    with tile.TileContext(nc) as tc:
        with (
            tc.tile_pool(name="const", bufs=1) as cpool,
            tc.tile_pool(name="sbuf", bufs=2) as pool,
            tc.tile_pool(name="hbuf", bufs=2) as hpool,
            tc.tile_pool(name="psum", bufs=2, space="PSUM") as psum,
            tc.tile_pool(name="opsum", bufs=1, space="PSUM") as opsum,
        ):
            ident = cpool.tile([P, P], F32)
            make_identity(nc, ident[:])
            w1k_t = cpool.tile([C, H], F32)
            nc.sync.dma_start(out=w1k_t[:], in_=w1k[:])
            b1_t = cpool.tile([H, 1], F32)
            nc.sync.dma_start(out=b1_t[:], in_=b1[:])
            w2_t = cpool.tile([H, 1], F32)
            nc.sync.dma_start(out=w2_t[:], in_=w2[:])
            b2_t = cpool.tile([1, 1], F32)
            nc.sync.dma_start(out=b2_t[:], in_=b2[:])

            for g in range(0, NT, OG):
                ng = min(OG, NT - g)
                h_g = hpool.tile([H, OG * P], F32, tag="h_g")
                o_ps = opsum.tile([1, OG * P], F32, space="PSUM", tag="o_ps")
                y_sb = hpool.tile([1, OG * P], F32, tag="y_sb")
                for ti in range(ng):
                    t = g + ti
                    idx_t = pool.tile([P, KS], I32, tag="idx")
                    nc.sync.dma_start(out=idx_t[:], in_=idx[t * P : (t + 1) * P, :])
                    xs = pool.tile([P, KS * C], F32, tag="xs")
                    for j in range(KS):
                        nc.gpsimd.indirect_dma_start(
                            out=xs[:, j * C : (j + 1) * C],
                            out_offset=None,
                            in_=x[:],
                            in_offset=bass.IndirectOffsetOnAxis(
                                ap=idx_t[:, j : j + 1], axis=0
                            ),
                        )
                    xd_b = xs[:, (KS - 1) * C : KS * C][:, None, :].to_broadcast(
                        [P, KS, C]
                    )
                    nc.vector.tensor_tensor(
                        out=xs[:], in0=xs[:], in1=xd_b,
                        op=mybir.AluOpType.subtract,
                    )
                    nc.scalar.activation(
                        xs[:], xs[:], mybir.ActivationFunctionType.Square
                    )
                    v = pool.tile([P, C], F32, tag="v")
                    nc.vector.reduce_sum(
                        out=v[:],
                        in_=xs[:].rearrange("p (j c) -> p c j", j=KS),
                        axis=mybir.AxisListType.X,
                    )
                    vt_ps = psum.tile([C, P], F32, space="PSUM", tag="vt")
                    nc.tensor.transpose(out=vt_ps[:], in_=v[:], identity=ident[:])
                    vt = pool.tile([C, P], F32, tag="vts")
                    nc.vector.tensor_copy(out=vt[:], in_=vt_ps[:])
                    h_ps = psum.tile([H, P], F32, space="PSUM", tag="h_ps")
                    nc.tensor.matmul(
                        out=h_ps[:], lhsT=w1k_t[:], rhs=vt[:], start=True, stop=True
                    )
                    nc.scalar.activation(
                        h_g[:, ti * P : (ti + 1) * P],
                        h_ps[:],
                        mybir.ActivationFunctionType.Relu,
                        bias=b1_t[:, :1],
                    )
                for s in range(0, ng * P, 512):
                    e = min(s + 512, ng * P)
                    nc.tensor.matmul(
                        out=o_ps[:, s:e], lhsT=w2_t[:], rhs=h_g[:, s:e],
                        start=True, stop=True,
                    )
                nc.scalar.activation(
                    y_sb[:, : ng * P],
                    o_ps[:, : ng * P],
                    mybir.ActivationFunctionType.Sigmoid,
                    bias=b2_t[:, :1],
                )
                nc.sync.dma_start(
                    out=y[:, g * P : g * P + ng * P], in_=y_sb[:, : ng * P]
                )
    nc.compile()
    return nc


# ------------------------------------------------------------------ driver


def _mlp_consts(W1, b1, W2, b2, k, H):
    kk = float(np.asarray(k))
    return (
        np.ascontiguousarray(np.asarray(W1, dtype=np.float32) / kk),
        np.ascontiguousarray(np.asarray(b1, dtype=np.float32).reshape(H, 1)),
        np.ascontiguousarray(np.asarray(W2, dtype=np.float32).reshape(H, 1)),
        np.ascontiguousarray(np.asarray(b2, dtype=np.float32).reshape(1, 1)),
    )


def _run_indirect(x, src, dst, k, W1, b1, W2, b2):
    from concourse.bass_utils import run_bass_kernel_spmd

    N, C = x.shape
    H = W1.shape[1]
    starts, nbrs, deg = _node_lists(x, np.asarray(src), np.asarray(dst), k)
    K = nbrs.shape[1]
    KS = K + 1
    n_cores = 8
    per_core = max(starts[c + 1] - starts[c] for c in range(n_cores))
    NT = (per_core + P - 1) // P
    n_nodes_pad = NT * P

    key = ("ind", N, C, KS, NT, n_nodes_pad, H)
    if key not in _KERNEL_CACHE:
        _KERNEL_CACHE[key] = _build_indirect(N, C, KS, NT, n_nodes_pad, H)
    nc = _KERNEL_CACHE[key]

    w1k, b1v, w2v, b2v = _mlp_consts(W1, b1, W2, b2, k, H)

    in_maps = []
    for c in range(n_cores):
        lo, hi = starts[c], starts[c + 1]
        idx = np.zeros((n_nodes_pad, KS), dtype=np.int32)
        nb = nbrs[lo:hi]
        own = np.arange(lo, hi, dtype=np.int64)
        nb2 = np.where(nb >= 0, nb, own[:, None])
        idx[: hi - lo, :K] = nb2
        idx[: hi - lo, K] = own
        in_maps.append(
            {"x": x, "idx": idx, "w1k": w1k, "b1": b1v, "w2": w2v, "b2": b2v}
        )

    res = run_bass_kernel_spmd(nc, in_maps, core_ids=list(range(n_cores)))
    out = np.empty((N, 1), dtype=np.float32)
    for c in range(n_cores):
        lo, hi = starts[c], starts[c + 1]
        out[lo:hi, 0] = res.results[c]["y"][0, : hi - lo]
    return out


def _run_gather(x, src, dst, k, W1, b1, W2, b2):
    from concourse.bass_utils import run_bass_kernel_spmd

    N, C = x.shape
    H = W1.shape[1]
    plan = _plan_all(x, np.asarray(src), np.asarray(dst), k)
    n_cores = 8
    n_nodes_pad = plan["n_nodes_pad"]
    starts = plan["starts"]

    key = ("gat", N, C, plan["NT"], n_nodes_pad, H, plan["A"],
           tuple(plan["OVC"].ravel().tolist()))
    if key not in _KERNEL_CACHE:
        _KERNEL_CACHE[key] = _build_gather(plan, H)
    nc = _KERNEL_CACHE[key]

    w1k, b1v, w2v, b2v = _mlp_consts(W1, b1, W2, b2, k, H)
    iota = np.tile(np.arange(P, dtype=np.float32), (P, 1))
    xq = _build_x_aug(x)

    in_maps = []
    for c in range(n_cores):
        lo, hi = starts[c], starts[c + 1]
        xl = np.zeros((n_nodes_pad, C), dtype=np.float32)
        xl[: hi - lo] = x[lo:hi]
        sd = np.zeros((n_nodes_pad, 1), dtype=np.float32)
        sd[: hi - lo, 0] = np.sqrt(plan["deg"][lo:hi].astype(np.float32))
        m = {
            "idx16": plan["core_data"][c]["idx16"],
            "ovt": plan["core_data"][c]["ovt"],
            "xloc": xl,
            "sdeg": sd,
            "iota": iota,
            "w1k": w1k,
            "b1": b1v,
            "w2": w2v,
            "b2": b2v,
        }
        for q in range(NCLASS):
            m[f"xq{q}"] = xq[q]
        in_maps.append(m)

    res = run_bass_kernel_spmd(nc, in_maps, core_ids=list(range(n_cores)))
    out = np.empty((N, 1), dtype=np.float32)
    for c in range(n_cores):
        lo, hi = starts[c], starts[c + 1]
        out[lo:hi, 0] = res.results[c]["y"][0, : hi - lo]
    return out


def kernel(x, src, dst, k, W1, b1, W2, b2):
    global CHUNK
    _install_ntff_hook()
    x = np.ascontiguousarray(np.asarray(x, dtype=np.float32))
    N = x.shape[0]
    chunk = -(-N // NCLASS)  # ceil
    if MODE == "gather" and chunk <= 32767 and x.shape[1] == 128:
        CHUNK = chunk
        return _run_gather(x, src, dst, k, W1, b1, W2, b2)
    return _run_indirect(x, src, dst, k, W1, b1, W2, b2)


def run_traced(**inputs):
    """test.py helper: run with NTFF tracing, return (output, exec_time_ns)."""
    _install_ntff_hook()
    import concourse.bass_utils as bu

    orig = bu.run_bass_kernel_spmd
    holder = {}

    def wrapper(nc, in_maps, core_ids, **kw):
        kw["trace"] = True
        r = orig(nc, in_maps, core_ids, **kw)
        holder["exec_time_ns"] = r.exec_time_ns
        return r

    bu.run_bass_kernel_spmd = wrapper
    try:
        out = kernel(**inputs)
    finally:
        bu.run_bass_kernel_spmd = orig
    return out, holder.get("exec_time_ns")



# revision 12
# speedup vs baseline: 1.3659x; 1.3659x over previous
"""Trainium2 Bass kernel for nn_BoundaryAttentionHead (gnn_message_passing).

reference computation:
    diff = (x[src] - x[dst])**2                    # [E, C]
    variance = scatter_add(diff, dst) / k          # [N, C]
    h = relu(variance @ W1 + b1)                   # [N, 64]
    out = sigmoid(h @ W2 + b2)                     # [N, 1]

Sharding: nodes across 8 cores (dst-segment partition); MLP weights
replicated; W1 pre-scaled by 1/k on host.

Per-node expansion (avoids per-edge subtraction):
    sum_j (x_sj - x_n)^2 = S2 - 2 x_n . S1 + deg * x_n^2
    S1 = sum_j x_sj,  S2 = sum_j x_sj^2

"gather" mode: batched SWDGE dma_gather (int16 indices) over four
32768-row windows of a zero-row-augmented bf16 copy of x, spread over 4
SWDGE queues. Per 128-node tile and window class q: A aligned columns
(node-major; holes -> zero row) + overflow columns (dense packing of the
>A leftovers) routed back to their node with a one-hot (Sel) matmul on PE.
Gathered data and the reduction tree are bf16 (256B descriptors, 2x DVE);
final accumulations in f32.

"indirect" mode (BAH_MODE=indirect): simple fallback, one indirect DMA per
(tile, slot); ~4.7x slower but trivially correct.
"""
import os
import sys
import types

import numpy as np

_KERNEL_CACHE = {}
MODE = os.environ.get("BAH_MODE", "gather")

P = 128
NCLASS = 4
CHUNK = 25000
WIN = 32768
GT = 5  # tiles per group
ALIGN_A = 2
GRUN = int(os.environ.get("BAH_GRUN", "24"))  # max cols per dma_gather call


def _install_ntff_hook():
    if "antenv.axon_hooks" in sys.modules:
        return
    sys.path.insert(0, "/root/.axon_site")
    try:
        from trn_agent_boot.trn_boot import _ntff_profile_via_ctypes
    except Exception:
        return
    mod = types.ModuleType("antenv.axon_hooks")
    _hook = [_ntff_profile_via_ctypes("/opt/axon/libaxon_pjrt.so")]
    mod.get_axon_ntff_profile_hook = lambda: _hook[0]
    mod.set_axon_ntff_profile_hook = lambda h: _hook.__setitem__(0, h)
    sys.modules["antenv.axon_hooks"] = mod


# ---------------------------------------------------------------- host side


def _node_lists(x, src, dst, k):
    """Per-node neighbour lists (-1 padded), degrees, per-core node ranges."""
    N, C = x.shape
    E = src.shape[0]
    n_cores = 8
    base = N // n_cores
    rem = N % n_cores
    starts = [c * base + min(c, rem) for c in range(n_cores)] + [N]

    src = np.asarray(src)
    dst = np.asarray(dst)

    fast = False
    if E % N == 0 and E // N > 0:
        K = E // N
        fast = np.array_equal(dst, np.repeat(np.arange(N, dtype=dst.dtype), K))

    if fast:
        nbrs = src.reshape(N, E // N).astype(np.int64)
        deg = np.full(N, E // N, dtype=np.int64)
    else:
        order = np.argsort(dst, kind="stable")
        ds = dst[order].astype(np.int64)
        ss = src[order].astype(np.int64)
        deg = np.bincount(ds, minlength=N)
        Kmax = int(deg.max()) if E else 1
        nbrs = np.full((N, Kmax), -1, dtype=np.int64)
        seg_off = np.zeros(N + 1, dtype=np.int64)
        np.cumsum(deg, out=seg_off[1:])
        pos = np.arange(E, dtype=np.int64) - seg_off[ds]
        nbrs[ds, pos] = ss
    return starts, nbrs, deg


def _build_x_aug(x):
    """Four [WIN, C] bf16 tables: row 0 zero, rows 1.. = x[CHUNK*q:...+WIN-1]."""
    import ml_dtypes

    N, C = x.shape
    tabs = []
    for q in range(NCLASS):
        t = np.zeros((WIN, C), dtype=ml_dtypes.bfloat16)
        lo = CHUNK * q
        hi = min(N, lo + WIN - 1)
        if hi > lo:
            t[1 : 1 + hi - lo] = np.asarray(x[lo:hi], dtype=np.float32).astype(
                ml_dtypes.bfloat16
            )
        tabs.append(t)
    return tabs


def _plan_core(nbrs_core, A, NT):
    """Plan one core. Returns (aligned [NT,4,A,128] i16,
    ov: {(t,q): (refs i16 [m*128], tgts i32 [m*128])}, ov_needed [NT,4])."""
    aligned = np.zeros((NT, NCLASS, A, P), dtype=np.int16)
    ov = {}
    ov_needed = np.zeros((NT, NCLASS), dtype=np.int64)
    n = nbrs_core.shape[0]
    for t in range(NT):
        blk = nbrs_core[t * P : min(n, (t + 1) * P)]  # [npn, K]
        npn = blk.shape[0]
        for q in range(NCLASS):
            sel = (blk >= CHUNK * q) & (blk < CHUNK * (q + 1))
            rank = np.cumsum(sel, axis=1) - 1  # within-row rank where sel
            refs = (blk - CHUNK * q + 1).astype(np.int32)
            # aligned part
            am = sel & (rank < A)
            pp, jj = np.nonzero(am)
            aligned[t, q, rank[pp, jj], pp] = refs[pp, jj].astype(np.int16)
            # overflow part (row-major nonzero => grouped by p)
            om = sel & (rank >= A)
            po, jo = np.nonzero(om)
            cnt = len(po)
            m = (cnt + P - 1) // P
            ov_needed[t, q] = m
            if m:
                orf = np.zeros(m * P, dtype=np.int16)
                otg = np.full(m * P, 200, dtype=np.int32)
                orf[:cnt] = refs[po, jo].astype(np.int16)
                otg[:cnt] = po
                ov[(t, q)] = (orf, otg)
    return aligned, ov, ov_needed


def _wrap_call(vals):
    """Pack one column's 128 int16 values into its [128, 8] SBUF idx block:
    call-flat element i -> (i%16, i//16), replicated x8 down partitions."""
    a = np.zeros((16, 8), dtype=np.int16)
    i = np.arange(P)
    a[i % 16, i // 16] = vals
    return np.tile(a, (8, 1))


def _layout_groups(NT, OVC, A):
    """Class blocks are padded to a uniform per-group width so a tile's
    aligned columns across all 4 classes form a regular 4D access pattern
    (enables a single DVE reduce per tile)."""
    groups = []
    for g0 in range(0, NT, GT):
        tiles = list(range(g0, min(NT, g0 + GT)))
        ngt = len(tiles)
        col = 0
        qblocks = []
        acol = {}
        ocol = {}
        ovlist = []
        for q in range(NCLASS):
            qstart = col
            for t in tiles:
                acol[(t, q)] = col
                col += A
            for t in tiles:
                ocol[(t, q)] = col
                for cc in range(int(OVC[t, q])):
                    ovlist.append((t, q, cc))
                col += int(OVC[t, q])
            qblocks.append((qstart, col))
        groups.append(
            dict(tiles=tiles, ngt=ngt, ncol=col, qblocks=qblocks,
                 acol=acol, ocol=ocol, ovlist=ovlist)
        )
    return groups


def _plan_all(x, src, dst, k, A=ALIGN_A):
    N, C = x.shape
    starts, nbrs, deg = _node_lists(x, src, dst, k)
    n_cores = 8
    per_core = max(starts[c + 1] - starts[c] for c in range(n_cores))
    NT = (per_core + P - 1) // P
    n_nodes_pad = NT * P

    cores = []
    for c in range(n_cores):
        lo, hi = starts[c], starts[c + 1]
        nb = np.full((n_nodes_pad, nbrs.shape[1]), -1, dtype=np.int64)
        nb[: hi - lo] = nbrs[lo:hi]
        cores.append(_plan_core(nb, A, NT))

    OVC = np.zeros((NT, NCLASS), dtype=np.int64)
    for _, _, ovn in cores:
        OVC = np.maximum(OVC, ovn)

    groups = _layout_groups(NT, OVC, A)
    total_cols = sum(g["ncol"] for g in groups)
    total_ovcols = sum(len(g["ovlist"]) for g in groups)

    import ml_dtypes

    core_data = []
    for c in range(n_cores):
        al, ov, _ = cores[c]
        idx16 = np.zeros((P, total_cols * 8), dtype=np.int16)
        ovt = np.full((P, max(total_ovcols, 1)), 200.0, dtype=ml_dtypes.bfloat16)
        colbase = 0
        ovbase = 0
        for g in groups:
            for q in range(NCLASS):
                for t in g["tiles"]:
                    a0 = g["acol"][(t, q)]
                    for a in range(A):
                        j = colbase + a0 + a
                        idx16[:, j * 8 : (j + 1) * 8] = _wrap_call(al[t, q, a])
            for li, (t, q, cc) in enumerate(g["ovlist"]):
                j = colbase + g["ocol"][(t, q)] + cc
                rr, tt = ov.get((t, q), (None, None))
                if rr is None:
                    vals = np.zeros(P, dtype=np.int16)
                    tgts = np.full(P, 200, dtype=np.int32)
                else:
                    if len(rr) < (cc + 1) * P:
                        rr = np.concatenate(
                            [rr, np.zeros((cc + 1) * P - len(rr), np.int16)]
                        )
                        tt = np.concatenate(
                            [tt, np.full((cc + 1) * P - len(tt), 200, np.int32)]
                        )
                    vals = rr[cc * P : (cc + 1) * P]
                    tgts = tt[cc * P : (cc + 1) * P]
                idx16[:, j * 8 : (j + 1) * 8] = _wrap_call(vals)
                ovt[:, ovbase + li] = tgts.astype(np.float32)
            colbase += g["ncol"]
            ovbase += len(g["ovlist"])
        core_data.append(dict(idx16=idx16, ovt=ovt))

    return dict(
        N=N, C=C, NT=NT, n_nodes_pad=n_nodes_pad, A=A,
        starts=starts, deg=deg, OVC=OVC, groups=groups,
        total_cols=total_cols, total_ovcols=total_ovcols,
        core_data=core_data,
    )


# ------------------------------------------------------------- device side


def _build_gather(plan, H):
    import concourse.bacc as bacc
    import concourse.tile as tile
    from concourse import mybir
    from concourse.library_config import mlp
    from concourse.masks import make_identity

    F32 = mybir.dt.float32
    BF16 = mybir.dt.bfloat16
    I16 = mybir.dt.int16
    C = plan["C"]
    A = plan["A"]
    groups = plan["groups"]
    n_nodes_pad = plan["n_nodes_pad"]

    nc = bacc.Bacc("TRN2", num_swdge_queues=4, dynamic_dma_scratch_size=32768)
    xq_t = [
        nc.dram_tensor(f"xq{q}", [WIN, C], BF16, kind="ExternalInput")
        for q in range(NCLASS)
    ]
    idx16 = nc.dram_tensor(
        "idx16", [P, plan["total_cols"] * 8], I16, kind="ExternalInput"
    )
    ovt_d = nc.dram_tensor(
        "ovt", [P, max(plan["total_ovcols"], 1)], BF16, kind="ExternalInput"
    )
    xloc = nc.dram_tensor("xloc", [n_nodes_pad, C], F32, kind="ExternalInput")
    sdeg = nc.dram_tensor("sdeg", [n_nodes_pad, 1], F32, kind="ExternalInput")
    iota = nc.dram_tensor("iota", [P, P], BF16, kind="ExternalInput")
    w1k = nc.dram_tensor("w1k", [C, H], F32, kind="ExternalInput")
    b1 = nc.dram_tensor("b1", [H, 1], F32, kind="ExternalInput")
    w2 = nc.dram_tensor("w2", [H, 1], F32, kind="ExternalInput")
    b2 = nc.dram_tensor("b2", [1, 1], F32, kind="ExternalInput")
    y = nc.dram_tensor("y", [1, n_nodes_pad], F32, kind="ExternalOutput")

    qrr = [0]  # SWDGE queue round-robin

    with tile.TileContext(nc) as tc:
        with tc.tile_critical():
            nc.gpsimd.load_library(mlp)
        with (
            tc.tile_pool(name="const", bufs=1) as cpool,
            tc.tile_pool(name="grp", bufs=4) as gpool,
            tc.tile_pool(name="spool", bufs=2 * GT + 2) as spool,
            tc.tile_pool(name="selp", bufs=1) as selp,
            tc.tile_pool(name="sbuf", bufs=2) as pool,
            tc.tile_pool(name="hbuf", bufs=1) as hpool,
            tc.tile_pool(name="psum", bufs=2, space="PSUM") as psum,
            tc.tile_pool(name="opsum", bufs=1, space="PSUM") as opsum,
        ):
            ident = cpool.tile([P, P], F32)
            make_identity(nc, ident[:])
            iota_t = cpool.tile([P, P], BF16)
            nc.sync.dma_start(out=iota_t[:], in_=iota[:])
            w1k_t = cpool.tile([C, H], F32)
            nc.sync.dma_start(out=w1k_t[:], in_=w1k[:])
            b1_t = cpool.tile([H, 1], F32)
            nc.sync.dma_start(out=b1_t[:], in_=b1[:])
            w2_t = cpool.tile([H, 1], F32)
            nc.sync.dma_start(out=w2_t[:], in_=w2[:])
            b2_t = cpool.tile([1, 1], F32)
            nc.sync.dma_start(out=b2_t[:], in_=b2[:])

            cum_col = [0]
            cum_ov = [0]
            for g in groups:
                cum_col.append(cum_col[-1] + g["ncol"])
                cum_ov.append(cum_ov[-1] + len(g["ovlist"]))

            ctxs = {}

            def emit_gathers(gi):
                g = groups[gi]
                ncol = g["ncol"]
                ovlist = g["ovlist"]
                novc = len(ovlist)
                ctx = {}
                grp = gpool.tile([P, ncol * C], BF16, tag="grp")
                grpv = grp[:].rearrange("p (t c) -> p t c", c=C)
                idxg = pool.tile([P, ncol * 8], I16, tag="idxg")
                nc.sync.dma_start(
                    out=idxg[:],
                    in_=idx16[:, cum_col[gi] * 8 : (cum_col[gi] + ncol) * 8],
                )
                for q in range(NCLASS):
                    qs, qe = g["qblocks"][q]
                    c0 = qs
                    while c0 < qe:
                        run = min(GRUN, qe - c0)
                        n = run * P
                        nc.gpsimd.dma_gather(
                            grpv[:, c0 : c0 + run, :],
                            xq_t[q][:],
                            idxg[:, c0 * 8 : (c0 + run) * 8],
                            n, n, C,
                            queue_num=qrr[0] % 4,
                        )
                        qrr[0] += 1
                        c0 += run
                ctx["grp"] = grp
                ctx["grpv"] = grpv
                if novc:
                    ovtg = pool.tile([P, novc], BF16, tag="ovtg")
                    nc.sync.dma_start(
                        out=ovtg[:],
                        in_=ovt_d[:, cum_ov[gi] : cum_ov[gi] + novc],
                    )
                    sel_g = selp.tile([P, novc * P], BF16, tag="sel_g")
                    nc.vector.tensor_tensor(
                        out=sel_g[:].rearrange("p (o n) -> p o n", n=P),
                        in0=ovtg[:, :, None].to_broadcast([P, novc, P]),
                        in1=iota_t[:, None, :].to_broadcast([P, novc, P]),
                        op=mybir.AluOpType.is_equal,
                    )
                    ctx["sel_g"] = sel_g
                    ctx["ov_local"] = {
                        (t, q, cc): li for li, (t, q, cc) in enumerate(ovlist)
                    }
                ctxs[gi] = ctx

            def emit_compute(gi):
                g = groups[gi]
                ngt = g["ngt"]
                ovlist = g["ovlist"]
                novc = len(ovlist)
                ctx = ctxs.pop(gi)
                grp = ctx["grp"]
                grpv = ctx["grpv"]

                if novc:
                    sel_g = ctx["sel_g"]
                    ov_local = ctx["ov_local"]
                    ovbf = gpool.tile([P, novc * 2 * C], BF16, tag="ovbf")
                    ovbfv = ovbf[:].rearrange("p (t c) -> p t c", c=2 * C)
                    for q in range(NCLASS):
                        tq = [e for e in ovlist if e[1] == q]
                        if not tq:
                            continue
                        li0 = ov_local[tq[0]]
                        ov0 = g["ocol"][(tq[0][0], q)]
                        ovq = len(tq)
                        src_ap = grpv[:, ov0 : ov0 + ovq, :]
                        nc.vector.tensor_copy(
                            out=ovbfv[:, li0 : li0 + ovq, 0:C], in_=src_ap
                        )
                        nc.scalar.activation(
                            ovbfv[:, li0 : li0 + ovq, C : 2 * C],
                            src_ap,
                            mybir.ActivationFunctionType.Square,
                        )

                def _pairsum(t, tag):
                    blks = [
                        grp[:, g["acol"][(t, q)] * C : (g["acol"][(t, q)] + A) * C]
                        for q in range(NCLASS)
                    ]
                    t1 = pool.tile([P, A * C], BF16, tag=tag + "a")
                    t2 = pool.tile([P, A * C], BF16, tag=tag + "b")
                    nc.vector.tensor_add(out=t1[:], in0=blks[0], in1=blks[1])
                    nc.vector.tensor_add(out=t2[:], in0=blks[2], in1=blks[3])
                    t3 = pool.tile([P, A * C], F32, tag=tag + "c")
                    nc.vector.tensor_add(out=t3[:], in0=t1[:], in1=t2[:])
                    s = spool.tile([P, C], F32, tag=tag)
                    nc.vector.tensor_add(
                        out=s[:], in0=t3[:, 0:C], in1=t3[:, C : A * C]
                    )
                    return s

                s1_list = {}
                s2_list = {}
                for t in g["tiles"]:
                    s1_list[t] = _pairsum(t, "s1")
                # squares in place: classes 0,1 on ACT; 2,3 on DVE
                for q in range(NCLASS):
                    a0 = g["acol"][(g["tiles"][0], q)]
                    ap = grp[:, a0 * C : (a0 + ngt * A) * C]
                    if q < 2:
                        nc.scalar.activation(
                            ap, ap, mybir.ActivationFunctionType.Square
                        )
                    else:
                        nc.vector.tensor_tensor(
                            out=ap, in0=ap, in1=ap, op=mybir.AluOpType.mult
                        )
                for t in g["tiles"]:
                    s2_list[t] = _pairsum(t, "s2")

                h_g = hpool.tile([H, GT * P], F32, tag="h_g")
                o_ps = opsum.tile([1, GT * P], F32, space="PSUM", tag="o_ps")
                y_sb = hpool.tile([1, GT * P], F32, tag="y_sb")

                for ti, t in enumerate(g["tiles"]):
                    s1 = s1_list[t]
                    s2 = s2_list[t]
                    mmlist = [
                        (q, cc)
                        for q in range(NCLASS)
                        for cc in range(int(plan["OVC"][t, q]))
                    ]
                    ovps = None
                    if mmlist:
                        ovps = psum.tile([P, 2 * C], F32, space="PSUM", tag="ovps")
                        for mi, (q, cc) in enumerate(mmlist):
                            li = ov_local[(t, q, cc)]
                            nc.tensor.matmul(
                                out=ovps[:],
                                lhsT=sel_g[:, li * P : (li + 1) * P],
                                rhs=ovbfv[:, li, :],
                                start=(mi == 0),
                                stop=(mi == len(mmlist) - 1),
                            )
                    xl = pool.tile([P, C], F32, tag="xl")
                    nc.sync.dma_start(out=xl[:], in_=xloc[t * P : (t + 1) * P, :])
                    sd = pool.tile([P, 1], F32, tag="sd")
                    nc.sync.dma_start(out=sd[:], in_=sdeg[t * P : (t + 1) * P, :])
                    if ovps is not None:
                        s1t = pool.tile([P, C], F32, tag="s1t")
                        s2t = pool.tile([P, C], F32, tag="s2t")
                        nc.vector.tensor_add(out=s1t[:], in0=s1[:], in1=ovps[:, 0:C])
                        nc.vector.tensor_add(
                            out=s2t[:], in0=s2[:], in1=ovps[:, C : 2 * C]
                        )
                    else:
                        s1t, s2t = s1, s2
                    # V = S2t + (-2 xl) . S1t + deg xl^2
                    xlm2 = pool.tile([P, C], F32, tag="xlm2")
                    nc.scalar.mul(out=xlm2[:], in_=xl[:], mul=-2.0)
                    m_t = pool.tile([P, C], F32, tag="m_t")
                    nc.vector.tensor_tensor(
                        out=m_t[:], in0=xlm2[:], in1=s1t[:],
                        op=mybir.AluOpType.mult,
                    )
                    u_t = pool.tile([P, C], F32, tag="u_t")
                    nc.scalar.activation(
                        u_t[:], xl[:], mybir.ActivationFunctionType.Square,
                        scale=sd[:, :1],
                    )
                    v_t = pool.tile([P, C], F32, tag="v_t")
                    nc.vector.tensor_add(out=v_t[:], in0=s2t[:], in1=m_t[:])
                    nc.vector.tensor_add(out=v_t[:], in0=v_t[:], in1=u_t[:])
                    vt_ps = psum.tile([C, P], F32, space="PSUM", tag="vt")
                    nc.tensor.transpose(out=vt_ps[:], in_=v_t[:], identity=ident[:])
                    vt = pool.tile([C, P], F32, tag="vts")
                    nc.scalar.copy(out=vt[:], in_=vt_ps[:])
                    h_ps = psum.tile([H, P], F32, space="PSUM", tag="h_ps")
                    nc.tensor.matmul(
                        out=h_ps[:], lhsT=w1k_t[:], rhs=vt[:], start=True, stop=True
                    )
                    nc.scalar.activation(
                        h_g[:, ti * P : (ti + 1) * P],
                        h_ps[:],
                        mybir.ActivationFunctionType.Relu,
                        bias=b1_t[:, :1],
                    )

                for s in range(0, ngt * P, 512):
                    e = min(s + 512, ngt * P)
                    nc.tensor.matmul(
                        out=o_ps[:, s:e], lhsT=w2_t[:], rhs=h_g[:, s:e],
                        start=True, stop=True,
                    )
                nc.scalar.activation(
                    y_sb[:, : ngt * P],
                    o_ps[:, : ngt * P],
                    mybir.ActivationFunctionType.Sigmoid,
                    bias=b2_t[:, :1],
                )
                g0 = g["tiles"][0]
                nc.sync.dma_start(
                    out=y[:, g0 * P : g0 * P + ngt * P], in_=y_sb[:, : ngt * P]
                )

            for gi in range(len(groups) + 1):
                if gi < len(groups):
                    emit_gathers(gi)
                if gi >= 1:
                    emit_compute(gi - 1)
    nc.compile()
    return nc


# ------------------------------------------------------- v1 fallback build


def _build_indirect(N, C, KS, NT, n_nodes_pad, H):
    import concourse.bass as bass
    import concourse.bacc as bacc
    import concourse.tile as tile
    from concourse import mybir
    from concourse.masks import make_identity

    F32 = mybir.dt.float32
    I32 = mybir.dt.int32
    OG = 8

    nc = bacc.Bacc("TRN2")
    x = nc.dram_tensor("x", [N, C], F32, kind="ExternalInput")
    idx = nc.dram_tensor("idx", [n_nodes_pad, KS], I32, kind="ExternalInput")
    w1k = nc.dram_tensor("w1k", [C, H], F32, kind="ExternalInput")
    b1 = nc.dram_tensor("b1", [H, 1], F32, kind="ExternalInput")
    w2 = nc.dram_tensor("w2", [H, 1], F32, kind="ExternalInput")
    b2 = nc.dram_tensor("b2", [1, 1], F32, kind="ExternalInput")
    y = nc.dram_tensor("y", [1, n_nodes_pad], F32, kind="ExternalOutput")

    with tile.TileContext(nc) as tc:
        with (
            tc.tile_pool(name="const", bufs=1) as cpool,
            tc.tile_pool(name="sbuf", bufs=2) as pool,
            tc.tile_pool(name="hbuf", bufs=2) as hpool,
            tc.tile_pool(name="psum", bufs=2, space="PSUM") as psum,
            tc.tile_pool(name="opsum", bufs=1, space="PSUM") as opsum,
        ):
            ident = cpool.tile([P, P], F32)
            make_identity(nc, ident[:])
            w1k_t = cpool.tile([C, H], F32)
            nc.sync.dma_start(out=w1k_t[:], in_=w1k[:])
            b1_t = cpool.tile([H, 1], F32)
            nc.sync.dma_start(out=b1_t[:], in_=b1[:])
            w2_t = cpool.tile([H, 1], F32)
            nc.sync.dma_start(out=w2_t[:], in_=w2[:])
            b2_t = cpool.tile([1, 1], F32)
            nc.sync.dma_start(out=b2_t[:], in_=b2[:])

            for g in range(0, NT, OG):
                ng = min(OG, NT - g)
                h_g = hpool.tile([H, OG * P], F32, tag="h_g")
                o_ps = opsum.tile([1, OG * P], F32, space="PSUM", tag="o_ps")
                y_sb = hpool.tile([1, OG * P], F32, tag="y_sb")
                for ti in range(ng):
                    t = g + ti
                    idx_t = pool.tile([P, KS], I32, tag="idx")
                    nc.sync.dma_start(out=idx_t[:], in_=idx[t * P : (t + 1) * P, :])
                    xs = pool.tile([P, KS * C], F32, tag="xs")
                    for j in range(KS):
                        nc.gpsimd.indirect_dma_start(
                            out=xs[:, j * C : (j + 1) * C],
                            out_offset=None,
                            in_=x[:],
                            in_offset=bass.IndirectOffsetOnAxis(
                                ap=idx_t[:, j : j + 1], axis=0
                            ),
                        )
                    xd_b = xs[:, (KS - 1) * C : KS * C][:, None, :].to_broadcast(
                        [P, KS, C]
                    )
                    nc.vector.tensor_tensor(
                        out=xs[:], in0=xs[:], in1=xd_b,
                        op=mybir.AluOpType.subtract,
                    )
                    nc.scalar.activation(
                        xs[:], xs[:], mybir.ActivationFunctionType.Square
                    )
                    v = pool.tile([P, C], F32, tag="v")
                    nc.vector.reduce_sum(
                        out=v[:],
                        in_=xs[:].rearrange("p (j c) -> p c j", j=KS),
                        axis=mybir.AxisListType.X,
                    )
                    vt_ps = psum.tile([C, P], F32, space="PSUM", tag="vt")
                    nc.tensor.transpose(out=vt_ps[:], in_=v[:], identity=ident[:])
                    vt = pool.tile([C, P], F32, tag="vts")
                    nc.vector.tensor_copy(out=vt[:], in_=vt_ps[:])
                    h_ps = psum.tile([H, P], F32, space="PSUM", tag="h_ps")
                    nc.tensor.matmul(
                        out=h_ps[:], lhsT=w1k_t[:], rhs=vt[:], start=True, stop=True
                    )
                    nc.scalar.activation(
                        h_g[:, ti * P : (ti + 1) * P],
                        h_ps[:],
                        mybir.ActivationFunctionType.Relu,
                        bias=b1_t[:, :1],
                    )
                for s in range(0, ng * P, 512):
                    e = min(s + 512, ng * P)
                    nc.tensor.matmul(
                        out=o_ps[:, s:e], lhsT=w2_t[:], rhs=h_g[:, s:e],
                        start=True, stop=True,
                    )
                nc.scalar.activation(
                    y_sb[:, : ng * P],
                    o_ps[:, : ng * P],
                    mybir.ActivationFunctionType.Sigmoid,
                    bias=b2_t[:, :1],
                )
                nc.sync.dma_start(
                    out=y[:, g * P : g * P + ng * P], in_=y_sb[:, : ng * P]
                )
    nc.compile()
    return nc


# ------------------------------------------------------------------ driver


def _mlp_consts(W1, b1, W2, b2, k, H):
    kk = float(np.asarray(k))
    return (
        np.ascontiguousarray(np.asarray(W1, dtype=np.float32) / kk),
        np.ascontiguousarray(np.asarray(b1, dtype=np.float32).reshape(H, 1)),
        np.ascontiguousarray(np.asarray(W2, dtype=np.float32).reshape(H, 1)),
        np.ascontiguousarray(np.asarray(b2, dtype=np.float32).reshape(1, 1)),
    )


def _run_indirect(x, src, dst, k, W1, b1, W2, b2):
    from concourse.bass_utils import run_bass_kernel_spmd

    N, C = x.shape
    H = W1.shape[1]
    starts, nbrs, deg = _node_lists(x, np.asarray(src), np.asarray(dst), k)
    K = nbrs.shape[1]
    KS = K + 1
    n_cores = 8
    per_core = max(starts[c + 1] - starts[c] for c in range(n_cores))
    NT = (per_core + P - 1) // P
    n_nodes_pad = NT * P

    key = ("ind", N, C, KS, NT, n_nodes_pad, H)
    if key not in _KERNEL_CACHE:
        _KERNEL_CACHE[key] = _build_indirect(N, C, KS, NT, n_nodes_pad, H)
    nc = _KERNEL_CACHE[key]

    w1k, b1v, w2v, b2v = _mlp_consts(W1, b1, W2, b2, k, H)

    in_maps = []
    for c in range(n_cores):
        lo, hi = starts[c], starts[c + 1]
        idx = np.zeros((n_nodes_pad, KS), dtype=np.int32)
        nb = nbrs[lo:hi]
        own = np.arange(lo, hi, dtype=np.int64)
        nb2 = np.where(nb >= 0, nb, own[:, None])
        idx[: hi - lo, :K] = nb2
        idx[: hi - lo, K] = own
        in_maps.append(
            {"x": x, "idx": idx, "w1k": w1k, "b1": b1v, "w2": w2v, "b2": b2v}
        )

    res = run_bass_kernel_spmd(nc, in_maps, core_ids=list(range(n_cores)))
    out = np.empty((N, 1), dtype=np.float32)
    for c in range(n_cores):
        lo, hi = starts[c], starts[c + 1]
        out[lo:hi, 0] = res.results[c]["y"][0, : hi - lo]
    return out


def _run_gather(x, src, dst, k, W1, b1, W2, b2):
    from concourse.bass_utils import run_bass_kernel_spmd

    N, C = x.shape
    H = W1.shape[1]
    plan = _plan_all(x, np.asarray(src), np.asarray(dst), k)
    n_cores = 8
    n_nodes_pad = plan["n_nodes_pad"]
    starts = plan["starts"]

    key = ("gat", N, C, plan["NT"], n_nodes_pad, H, plan["A"],
           tuple(plan["OVC"].ravel().tolist()))
    if key not in _KERNEL_CACHE:
        _KERNEL_CACHE[key] = _build_gather(plan, H)
    nc = _KERNEL_CACHE[key]

    w1k, b1v, w2v, b2v = _mlp_consts(W1, b1, W2, b2, k, H)
    import ml_dtypes

    iota = np.tile(np.arange(P, dtype=np.float32), (P, 1)).astype(ml_dtypes.bfloat16)
    xq = _build_x_aug(x)

    in_maps = []
    for c in range(n_cores):
        lo, hi = starts[c], starts[c + 1]
        xl = np.zeros((n_nodes_pad, C), dtype=np.float32)
        xl[: hi - lo] = x[lo:hi]
        sd = np.zeros((n_nodes_pad, 1), dtype=np.float32)
        sd[: hi - lo, 0] = np.sqrt(plan["deg"][lo:hi].astype(np.float32))
        m = {
            "idx16": plan["core_data"][c]["idx16"],
            "ovt": plan["core_data"][c]["ovt"],
            "xloc": xl,
            "sdeg": sd,
            "iota": iota,
            "w1k": w1k,
            "b1": b1v,
            "w2": w2v,
            "b2": b2v,
        }
        for q in range(NCLASS):
            m[f"xq{q}"] = xq[q]
        in_maps.append(m)

    res = run_bass_kernel_spmd(nc, in_maps, core_ids=list(range(n_cores)))
    out = np.empty((N, 1), dtype=np.float32)
    for c in range(n_cores):
        lo, hi = starts[c], starts[c + 1]
        out[lo:hi, 0] = res.results[c]["y"][0, : hi - lo]
    return out


def kernel(x, src, dst, k, W1, b1, W2, b2):
    global CHUNK
    _install_ntff_hook()
    x = np.ascontiguousarray(np.asarray(x, dtype=np.float32))
    N = x.shape[0]
    chunk = -(-N // NCLASS)  # ceil
    if MODE == "gather" and chunk <= 32767 and x.shape[1] == 128:
        CHUNK = chunk
        return _run_gather(x, src, dst, k, W1, b1, W2, b2)
    return _run_indirect(x, src, dst, k, W1, b1, W2, b2)


def run_traced(**inputs):
    """test.py helper: run with NTFF tracing, return (output, exec_time_ns)."""
    _install_ntff_hook()
    import concourse.bass_utils as bu

    orig = bu.run_bass_kernel_spmd
    holder = {}

    def wrapper(nc, in_maps, core_ids, **kw):
        kw["trace"] = True
        r = orig(nc, in_maps, core_ids, **kw)
        holder["exec_time_ns"] = r.exec_time_ns
        return r

    bu.run_bass_kernel_spmd = wrapper
    try:
        out = kernel(**inputs)
    finally:
        bu.run_bass_kernel_spmd = orig
    return out, holder.get("exec_time_ns")



# revision 21
# speedup vs baseline: 1.5035x; 1.1007x over previous
"""Trainium2 Bass kernel for nn_BoundaryAttentionHead (gnn_message_passing).

reference computation:
    diff = (x[src] - x[dst])**2                    # [E, C]
    variance = scatter_add(diff, dst) / k          # [N, C]
    h = relu(variance @ W1 + b1)                   # [N, 64]
    out = sigmoid(h @ W2 + b2)                     # [N, 1]

Sharding: nodes across 8 cores (dst-segment partition); MLP weights
replicated; W1 pre-scaled by 1/k on host.

Per-node expansion (avoids per-edge subtraction):
    sum_j (x_sj - x_n)^2 = S2 - 2 x_n . S1 + deg * x_n^2
    S1 = sum_j x_sj,  S2 = sum_j x_sj^2

"gather" mode: batched SWDGE dma_gather (int16 indices) over four
32768-row windows of a zero-row-augmented bf16 copy of x, spread over 4
SWDGE queues. Per 128-node tile and window class q: A aligned columns
(node-major; holes -> zero row) + overflow columns (dense packing of the
>A leftovers) routed back to their node with a one-hot (Sel) matmul on PE.
Gathered data and the reduction tree are bf16 (256B descriptors, 2x DVE);
final accumulations in f32.

"indirect" mode (BAH_MODE=indirect): simple fallback, one indirect DMA per
(tile, slot); ~4.7x slower but trivially correct.
"""
import os
import sys
import types

import numpy as np

_KERNEL_CACHE = {}
MODE = os.environ.get("BAH_MODE", "gather")

P = 128
NCLASS = 4
CHUNK = 25000
WIN = 32768
GT = 6  # tiles per group
ALIGN_A = 2
GRUN = int(os.environ.get("BAH_GRUN", "8"))  # max cols per dma_gather call
DMA_SCRATCH = int(os.environ.get("BAH_SCRATCH", "49152"))


def _install_ntff_hook():
    if "antenv.axon_hooks" in sys.modules:
        return
    sys.path.insert(0, "/root/.axon_site")
    try:
        from trn_agent_boot.trn_boot import _ntff_profile_via_ctypes
    except Exception:
        return
    mod = types.ModuleType("antenv.axon_hooks")
    _hook = [_ntff_profile_via_ctypes("/opt/axon/libaxon_pjrt.so")]
    mod.get_axon_ntff_profile_hook = lambda: _hook[0]
    mod.set_axon_ntff_profile_hook = lambda h: _hook.__setitem__(0, h)
    sys.modules["antenv.axon_hooks"] = mod


# ---------------------------------------------------------------- host side


def _node_lists(x, src, dst, k):
    """Per-node neighbour lists (-1 padded), degrees, per-core node ranges."""
    N, C = x.shape
    E = src.shape[0]
    n_cores = 8
    base = N // n_cores
    rem = N % n_cores
    starts = [c * base + min(c, rem) for c in range(n_cores)] + [N]

    src = np.asarray(src)
    dst = np.asarray(dst)

    fast = False
    if E % N == 0 and E // N > 0:
        K = E // N
        fast = np.array_equal(dst, np.repeat(np.arange(N, dtype=dst.dtype), K))

    if fast:
        nbrs = src.reshape(N, E // N).astype(np.int64)
        deg = np.full(N, E // N, dtype=np.int64)
    else:
        order = np.argsort(dst, kind="stable")
        ds = dst[order].astype(np.int64)
        ss = src[order].astype(np.int64)
        deg = np.bincount(ds, minlength=N)
        Kmax = int(deg.max()) if E else 1
        nbrs = np.full((N, Kmax), -1, dtype=np.int64)
        seg_off = np.zeros(N + 1, dtype=np.int64)
        np.cumsum(deg, out=seg_off[1:])
        pos = np.arange(E, dtype=np.int64) - seg_off[ds]
        nbrs[ds, pos] = ss
    return starts, nbrs, deg


def _build_x_aug(x):
    """Four [WIN, C] bf16 tables: row 0 zero, rows 1.. = x[CHUNK*q:...+WIN-1]."""
    import ml_dtypes

    N, C = x.shape
    tabs = []
    for q in range(NCLASS):
        t = np.zeros((WIN, C), dtype=ml_dtypes.bfloat16)
        lo = CHUNK * q
        hi = min(N, lo + WIN - 1)
        if hi > lo:
            t[1 : 1 + hi - lo] = np.asarray(x[lo:hi], dtype=np.float32).astype(
                ml_dtypes.bfloat16
            )
        tabs.append(t)
    return tabs


def _plan_core(nbrs_core, A, NT):
    """Plan one core. Returns (aligned [NT,4,A,128] i16,
    ov: {(t,q): (refs i16 [m*128], tgts i32 [m*128])}, ov_needed [NT,4])."""
    aligned = np.zeros((NT, NCLASS, A, P), dtype=np.int16)
    ov = {}
    ov_needed = np.zeros((NT, NCLASS), dtype=np.int64)
    n = nbrs_core.shape[0]
    for t in range(NT):
        blk = nbrs_core[t * P : min(n, (t + 1) * P)]  # [npn, K]
        npn = blk.shape[0]
        for q in range(NCLASS):
            sel = (blk >= CHUNK * q) & (blk < CHUNK * (q + 1))
            rank = np.cumsum(sel, axis=1) - 1  # within-row rank where sel
            refs = (blk - CHUNK * q + 1).astype(np.int32)
            # aligned part
            am = sel & (rank < A)
            pp, jj = np.nonzero(am)
            aligned[t, q, rank[pp, jj], pp] = refs[pp, jj].astype(np.int16)
            # overflow part (row-major nonzero => grouped by p)
            om = sel & (rank >= A)
            po, jo = np.nonzero(om)
            cnt = len(po)
            m = (cnt + P - 1) // P
            ov_needed[t, q] = m
            if m:
                orf = np.zeros(m * P, dtype=np.int16)
                otg = np.full(m * P, 200, dtype=np.int32)
                orf[:cnt] = refs[po, jo].astype(np.int16)
                otg[:cnt] = po
                ov[(t, q)] = (orf, otg)
    return aligned, ov, ov_needed


def _wrap_call(vals):
    """Pack one column's 128 int16 values into its [128, 8] SBUF idx block:
    call-flat element i -> (i%16, i//16), replicated x8 down partitions."""
    a = np.zeros((16, 8), dtype=np.int16)
    i = np.arange(P)
    a[i % 16, i // 16] = vals
    return np.tile(a, (8, 1))


def _layout_groups(NT, OVC, A):
    """Per-group column layout with a uniform per-class stride QS:
    class q occupies cols [q*QS, q*QS + ngt*A + ovc_q) -- ngt*A aligned
    cols (node-major) followed by that class's overflow cols, padded out
    to QS so the aligned blocks of all 4 classes sit at uniform stride
    (enables fused multi-class DVE reduction trees). Pad cols are never
    gathered."""
    groups = []
    for g0 in range(0, NT, GT):
        tiles = list(range(g0, min(NT, g0 + GT)))
        ngt = len(tiles)
        ovc_q = [int(sum(int(OVC[t, q]) for t in tiles)) for q in range(NCLASS)]
        OVG = max(ovc_q)
        QS = ngt * A + OVG
        acol = {}
        ocol = {}
        ovlist = []
        ranges = []
        for q in range(NCLASS):
            qbase = q * QS
            for ti, t in enumerate(tiles):
                acol[(t, q)] = qbase + ti * A
            col = qbase + ngt * A
            for t in tiles:
                ocol[(t, q)] = col
                for cc in range(int(OVC[t, q])):
                    ovlist.append((t, q, cc))
                col += int(OVC[t, q])
            ranges.append((qbase, col))
        groups.append(
            dict(tiles=tiles, ngt=ngt, ncol=NCLASS * QS, QS=QS, ovc_q=ovc_q,
                 acol=acol, ocol=ocol, ovlist=ovlist, ranges=ranges)
        )
    return groups


def _plan_all(x, src, dst, k, A=ALIGN_A):
    N, C = x.shape
    starts, nbrs, deg = _node_lists(x, src, dst, k)
    n_cores = 8
    per_core = max(starts[c + 1] - starts[c] for c in range(n_cores))
    NT = (per_core + P - 1) // P
    n_nodes_pad = NT * P

    cores = []
    for c in range(n_cores):
        lo, hi = starts[c], starts[c + 1]
        nb = np.full((n_nodes_pad, nbrs.shape[1]), -1, dtype=np.int64)
        nb[: hi - lo] = nbrs[lo:hi]
        cores.append(_plan_core(nb, A, NT))

    OVC = np.zeros((NT, NCLASS), dtype=np.int64)
    for _, _, ovn in cores:
        OVC = np.maximum(OVC, ovn)

    groups = _layout_groups(NT, OVC, A)
    total_cols = sum(g["ncol"] for g in groups)
    total_ovcols = sum(len(g["ovlist"]) for g in groups)

    import ml_dtypes

    core_data = []
    for c in range(n_cores):
        al, ov, _ = cores[c]
        idx16 = np.zeros((P, total_cols * 8), dtype=np.int16)
        ovt = np.full((P, max(total_ovcols, 1)), 200.0, dtype=ml_dtypes.bfloat16)
        colbase = 0
        ovbase = 0
        for g in groups:
            for q in range(NCLASS):
                for t in g["tiles"]:
                    a0 = g["acol"][(t, q)]
                    for a in range(A):
                        j = colbase + a0 + a
                        idx16[:, j * 8 : (j + 1) * 8] = _wrap_call(al[t, q, a])
            for li, (t, q, cc) in enumerate(g["ovlist"]):
                j = colbase + g["ocol"][(t, q)] + cc
                rr, tt = ov.get((t, q), (None, None))
                if rr is None:
                    vals = np.zeros(P, dtype=np.int16)
                    tgts = np.full(P, 200, dtype=np.int32)
                else:
                    if len(rr) < (cc + 1) * P:
                        rr = np.concatenate(
                            [rr, np.zeros((cc + 1) * P - len(rr), np.int16)]
                        )
                        tt = np.concatenate(
                            [tt, np.full((cc + 1) * P - len(tt), 200, np.int32)]
                        )
                    vals = rr[cc * P : (cc + 1) * P]
                    tgts = tt[cc * P : (cc + 1) * P]
                idx16[:, j * 8 : (j + 1) * 8] = _wrap_call(vals)
                ovt[:, ovbase + li] = tgts.astype(np.float32)
            colbase += g["ncol"]
            ovbase += len(g["ovlist"])
        core_data.append(dict(idx16=idx16, ovt=ovt))

    return dict(
        N=N, C=C, NT=NT, n_nodes_pad=n_nodes_pad, A=A,
        starts=starts, deg=deg, OVC=OVC, groups=groups,
        total_cols=total_cols, total_ovcols=total_ovcols,
        core_data=core_data,
    )


# ------------------------------------------------------------- device side


def _build_gather(plan, H):
    import concourse.bacc as bacc
    import concourse.tile as tile
    from concourse import mybir
    from concourse.library_config import mlp
    from concourse.masks import make_identity

    F32 = mybir.dt.float32
    BF16 = mybir.dt.bfloat16
    F16 = mybir.dt.float16
    I16 = mybir.dt.int16
    C = plan["C"]
    A = plan["A"]
    groups = plan["groups"]
    n_nodes_pad = plan["n_nodes_pad"]

    nc = bacc.Bacc(
        "TRN2", num_swdge_queues=4, dynamic_dma_scratch_size=DMA_SCRATCH
    )
    xq_t = [
        nc.dram_tensor(f"xq{q}", [WIN, C], BF16, kind="ExternalInput")
        for q in range(NCLASS)
    ]
    idx16 = nc.dram_tensor(
        "idx16", [P, plan["total_cols"] * 8], I16, kind="ExternalInput"
    )
    ovt_d = nc.dram_tensor(
        "ovt", [P, max(plan["total_ovcols"], 1)], BF16, kind="ExternalInput"
    )
    xm2 = nc.dram_tensor("xm2", [n_nodes_pad, C], F16, kind="ExternalInput")
    xu = nc.dram_tensor("xu", [n_nodes_pad, C], F16, kind="ExternalInput")
    iota = nc.dram_tensor("iota", [P, P], BF16, kind="ExternalInput")
    w1k = nc.dram_tensor("w1k", [C, H], BF16, kind="ExternalInput")
    b1 = nc.dram_tensor("b1", [H, 1], F32, kind="ExternalInput")
    w2 = nc.dram_tensor("w2", [H, 1], BF16, kind="ExternalInput")
    b2 = nc.dram_tensor("b2", [1, 1], F32, kind="ExternalInput")
    y = nc.dram_tensor("y", [1, n_nodes_pad], F32, kind="ExternalOutput")

    qrr = [0]  # SWDGE queue round-robin

    with tile.TileContext(nc) as tc:
        with tc.tile_critical():
            nc.gpsimd.load_library(mlp)
        with (
            tc.tile_pool(name="const", bufs=1) as cpool,
            tc.tile_pool(name="grp", bufs=3) as gpool,
            tc.tile_pool(name="ovp", bufs=2) as ovpool,
            tc.tile_pool(name="spool", bufs=2) as spool,
            tc.tile_pool(name="selp", bufs=2) as selp,
            tc.tile_pool(name="sbuf", bufs=2) as pool,
            tc.tile_pool(name="hbuf", bufs=2) as hpool,
            tc.tile_pool(name="psum", bufs=2, space="PSUM") as psum,
            tc.tile_pool(name="opsum", bufs=1, space="PSUM") as opsum,
        ):
            identb = cpool.tile([P, P], BF16)
            make_identity(nc, identb[:])
            iota_t = cpool.tile([P, P], BF16)
            nc.sync.dma_start(out=iota_t[:], in_=iota[:])
            w1k_t = cpool.tile([C, H], BF16)
            nc.sync.dma_start(out=w1k_t[:], in_=w1k[:])
            b1_t = cpool.tile([H, 1], F32)
            nc.sync.dma_start(out=b1_t[:], in_=b1[:])
            w2_t = cpool.tile([H, 1], BF16)
            nc.sync.dma_start(out=w2_t[:], in_=w2[:])
            b2_t = cpool.tile([1, 1], F32)
            nc.sync.dma_start(out=b2_t[:], in_=b2[:])

            cum_col = [0]
            cum_ov = [0]
            for g in groups:
                cum_col.append(cum_col[-1] + g["ncol"])
                cum_ov.append(cum_ov[-1] + len(g["ovlist"]))

            ctxs = {}

            def emit_gathers(gi):
                g = groups[gi]
                ncol = g["ncol"]
                ovlist = g["ovlist"]
                novc = len(ovlist)
                ctx = {}
                grp = gpool.tile([P, ncol * C], BF16, tag="grp")
                grpv = grp[:].rearrange("p (t c) -> p t c", c=C)
                idxg = pool.tile([P, ncol * 8], I16, tag="idxg")
                nc.sync.dma_start(
                    out=idxg[:],
                    in_=idx16[:, cum_col[gi] * 8 : (cum_col[gi] + ncol) * 8],
                )
                for q in range(NCLASS):
                    qs, qe = g["ranges"][q]
                    c0 = qs
                    while c0 < qe:
                        run = min(GRUN, qe - c0)
                        n = run * P
                        nc.gpsimd.dma_gather(
                            grpv[:, c0 : c0 + run, :],
                            xq_t[q][:],
                            idxg[:, c0 * 8 : (c0 + run) * 8],
                            n, n, C,
                            queue_num=qrr[0] % 4,
                        )
                        qrr[0] += 1
                        c0 += run
                ctx["grp"] = grp
                ctx["grpv"] = grpv
                if novc:
                    ovtg = pool.tile([P, novc], BF16, tag="ovtg")
                    nc.sync.dma_start(
                        out=ovtg[:],
                        in_=ovt_d[:, cum_ov[gi] : cum_ov[gi] + novc],
                    )
                    sel_g = selp.tile([P, novc * P], BF16, tag="sel_g")
                    nc.vector.tensor_tensor(
                        out=sel_g[:].rearrange("p (o n) -> p o n", n=P),
                        in0=ovtg[:, :, None].to_broadcast([P, novc, P]),
                        in1=iota_t[:, None, :].to_broadcast([P, novc, P]),
                        op=mybir.AluOpType.is_equal,
                    )
                    ctx["sel_g"] = sel_g
                    ctx["ov_local"] = {
                        (t, q, cc): li for li, (t, q, cc) in enumerate(ovlist)
                    }
                ctxs[gi] = ctx

            def emit_compute(gi):
                g = groups[gi]
                ngt = g["ngt"]
                QS = g["QS"]
                ovlist = g["ovlist"]
                novc = len(ovlist)
                ctx = ctxs.pop(gi)
                grp = ctx["grp"]
                grpv = ctx["grpv"]
                nW = ngt * C  # per-class aligned tree width at level t1

                # [x | x^2] rhs buffer for the overflow Sel matmuls
                if novc:
                    sel_g = ctx["sel_g"]
                    ov_local = ctx["ov_local"]
                    ovbf = ovpool.tile([P, novc * 2 * C], BF16, tag="ovbf")
                    ovbfv = ovbf[:].rearrange("p (t c) -> p t c", c=2 * C)
                    li0 = 0
                    for q in range(NCLASS):
                        ovq = g["ovc_q"][q]
                        if not ovq:
                            continue
                        o0 = q * QS + ngt * A
                        src_ap = grpv[:, o0 : o0 + ovq, :]
                        nc.scalar.copy(
                            out=ovbfv[:, li0 : li0 + ovq, 0:C], in_=src_ap
                        )
                        nc.vector.tensor_tensor(
                            out=ovbfv[:, li0 : li0 + ovq, C : 2 * C],
                            in0=src_ap, in1=src_ap, op=mybir.AluOpType.mult,
                        )
                        li0 += ovq

                # fused reduction tree: t1[q] = a0 + a1 per (q, tile)
                t1 = pool.tile([P, NCLASS * nW], BF16, tag="t1")

                def _tree(s_out):
                    for q in range(NCLASS):
                        qb = grp[:, q * QS * C : (q * QS + ngt * A) * C]
                        qbv = qb.rearrange("p (t a c) -> p t a c", a=A, c=C)
                        nc.vector.tensor_add(
                            out=t1[:, q * nW : (q + 1) * nW],
                            in0=qbv[:, :, 0, :], in1=qbv[:, :, 1, :],
                        )
                    nc.vector.tensor_add(
                        out=t1[:, 0 : 2 * nW],
                        in0=t1[:, 0 : 2 * nW], in1=t1[:, 2 * nW : 4 * nW],
                    )
                    nc.vector.tensor_add(
                        out=s_out[:], in0=t1[:, 0:nW], in1=t1[:, nW : 2 * nW]
                    )

                s1_all = spool.tile([P, nW], F32, tag="s1a")
                _tree(s1_all)
                # squares in place on the aligned blocks: 2 classes ACT, 2 DVE
                for q in range(NCLASS):
                    ap = grp[:, q * QS * C : (q * QS + ngt * A) * C]
                    if q < 2:
                        nc.scalar.activation(
                            ap, ap, mybir.ActivationFunctionType.Square
                        )
                    else:
                        nc.vector.tensor_tensor(
                            out=ap, in0=ap, in1=ap, op=mybir.AluOpType.mult
                        )
                s2_all = spool.tile([P, nW], F32, tag="s2a")
                _tree(s2_all)

                # merge overflow sums (PSUM) into s1/s2 in place, per tile
                h_g = hpool.tile([H, GT * P], BF16, tag="h_g")
                o_ps = opsum.tile([1, GT * P], F32, space="PSUM", tag="o_ps")
                y_sb = hpool.tile([1, GT * P], F32, tag="y_sb")

                for ti, t in enumerate(g["tiles"]):
                    mmlist = [
                        (q, cc)
                        for q in range(NCLASS)
                        for cc in range(int(plan["OVC"][t, q]))
                    ]
                    if mmlist:
                        ovps = psum.tile([P, 2 * C], F32, space="PSUM", tag="ovps")
                        for mi, (q, cc) in enumerate(mmlist):
                            li = ov_local[(t, q, cc)]
                            nc.tensor.matmul(
                                out=ovps[:],
                                lhsT=sel_g[:, li * P : (li + 1) * P],
                                rhs=ovbfv[:, li, :],
                                start=(mi == 0),
                                stop=(mi == len(mmlist) - 1),
                            )
                        cs = slice(ti * C, (ti + 1) * C)
                        nc.vector.tensor_add(
                            out=s1_all[:, cs], in0=s1_all[:, cs],
                            in1=ovps[:, 0:C],
                        )
                        nc.vector.tensor_add(
                            out=s2_all[:, cs], in0=s2_all[:, cs],
                            in1=ovps[:, C : 2 * C],
                        )

                # V = S2 + (-2 x) . S1 + deg x^2, group-wide
                g0 = g["tiles"][0]
                xm2_t = pool.tile([P, nW], F16, tag="xm2")
                nc.sync.dma_start(
                    out=xm2_t[:].rearrange("p (t c) -> p t c", c=C),
                    in_=xm2[g0 * P : (g0 + ngt) * P, :].rearrange(
                        "(t p) c -> p t c", p=P
                    ),
                )
                xu_t = pool.tile([P, nW], F16, tag="xu")
                nc.sync.dma_start(
                    out=xu_t[:].rearrange("p (t c) -> p t c", c=C),
                    in_=xu[g0 * P : (g0 + ngt) * P, :].rearrange(
                        "(t p) c -> p t c", p=P
                    ),
                )
                m_all = pool.tile([P, nW], F32, tag="m_all")
                nc.vector.tensor_tensor(
                    out=m_all[:], in0=xm2_t[:], in1=s1_all[:],
                    op=mybir.AluOpType.mult,
                )
                nc.vector.tensor_add(out=m_all[:], in0=m_all[:], in1=s2_all[:])
                v_all = pool.tile([P, nW], BF16, tag="v_all")
                nc.vector.tensor_add(out=v_all[:], in0=m_all[:], in1=xu_t[:])

                for ti, t in enumerate(g["tiles"]):
                    vt_ps = psum.tile([C, P], BF16, space="PSUM", tag="vt")
                    nc.tensor.transpose(
                        out=vt_ps[:], in_=v_all[:, ti * C : (ti + 1) * C],
                        identity=identb[:],
                    )
                    vt = pool.tile([C, P], BF16, tag="vts")
                    nc.scalar.copy(out=vt[:], in_=vt_ps[:])
                    h_ps = psum.tile([H, P], F32, space="PSUM", tag="h_ps")
                    nc.tensor.matmul(
                        out=h_ps[:], lhsT=w1k_t[:], rhs=vt[:], start=True, stop=True
                    )
                    nc.scalar.activation(
                        h_g[:, ti * P : (ti + 1) * P],
                        h_ps[:],
                        mybir.ActivationFunctionType.Relu,
                        bias=b1_t[:, :1],
                    )

                for s in range(0, ngt * P, 512):
                    e = min(s + 512, ngt * P)
                    nc.tensor.matmul(
                        out=o_ps[:, s:e], lhsT=w2_t[:], rhs=h_g[:, s:e],
                        start=True, stop=True,
                    )
                nc.scalar.activation(
                    y_sb[:, : ngt * P],
                    o_ps[:, : ngt * P],
                    mybir.ActivationFunctionType.Sigmoid,
                    bias=b2_t[:, :1],
                )
                nc.sync.dma_start(
                    out=y[:, g0 * P : g0 * P + ngt * P], in_=y_sb[:, : ngt * P]
                )

            for gi in range(len(groups) + 1):
                if gi < len(groups):
                    emit_gathers(gi)
                if gi >= 1:
                    emit_compute(gi - 1)
    nc.compile()
    return nc


# ------------------------------------------------------- v1 fallback build


def _build_indirect(N, C, KS, NT, n_nodes_pad, H):
    import concourse.bass as bass
    import concourse.bacc as bacc
    import concourse.tile as tile
    from concourse import mybir
    from concourse.masks import make_identity

    F32 = mybir.dt.float32
    I32 = mybir.dt.int32
    OG = 8

    nc = bacc.Bacc("TRN2")
    x = nc.dram_tensor("x", [N, C], F32, kind="ExternalInput")
    idx = nc.dram_tensor("idx", [n_nodes_pad, KS], I32, kind="ExternalInput")
    w1k = nc.dram_tensor("w1k", [C, H], F32, kind="ExternalInput")
    b1 = nc.dram_tensor("b1", [H, 1], F32, kind="ExternalInput")
    w2 = nc.dram_tensor("w2", [H, 1], F32, kind="ExternalInput")
    b2 = nc.dram_tensor("b2", [1, 1], F32, kind="ExternalInput")
    y = nc.dram_tensor("y", [1, n_nodes_pad], F32, kind="ExternalOutput")

    with tile.TileContext(nc) as tc:
        with (
            tc.tile_pool(name="const", bufs=1) as cpool,
            tc.tile_pool(name="sbuf", bufs=2) as pool,
            tc.tile_pool(name="hbuf", bufs=2) as hpool,
            tc.tile_pool(name="psum", bufs=2, space="PSUM") as psum,
            tc.tile_pool(name="opsum", bufs=1, space="PSUM") as opsum,
        ):
            ident = cpool.tile([P, P], F32)
            make_identity(nc, ident[:])
            w1k_t = cpool.tile([C, H], F32)
            nc.sync.dma_start(out=w1k_t[:], in_=w1k[:])
            b1_t = cpool.tile([H, 1], F32)
            nc.sync.dma_start(out=b1_t[:], in_=b1[:])
            w2_t = cpool.tile([H, 1], F32)
            nc.sync.dma_start(out=w2_t[:], in_=w2[:])
            b2_t = cpool.tile([1, 1], F32)
            nc.sync.dma_start(out=b2_t[:], in_=b2[:])

            for g in range(0, NT, OG):
                ng = min(OG, NT - g)
                h_g = hpool.tile([H, OG * P], F32, tag="h_g")
                o_ps = opsum.tile([1, OG * P], F32, space="PSUM", tag="o_ps")
                y_sb = hpool.tile([1, OG * P], F32, tag="y_sb")
                for ti in range(ng):
                    t = g + ti
                    idx_t = pool.tile([P, KS], I32, tag="idx")
                    nc.sync.dma_start(out=idx_t[:], in_=idx[t * P : (t + 1) * P, :])
                    xs = pool.tile([P, KS * C], F32, tag="xs")
                    for j in range(KS):
                        nc.gpsimd.indirect_dma_start(
                            out=xs[:, j * C : (j + 1) * C],
                            out_offset=None,
                            in_=x[:],
                            in_offset=bass.IndirectOffsetOnAxis(
                                ap=idx_t[:, j : j + 1], axis=0
                            ),
                        )
                    xd_b = xs[:, (KS - 1) * C : KS * C][:, None, :].to_broadcast(
                        [P, KS, C]
                    )
                    nc.vector.tensor_tensor(
                        out=xs[:], in0=xs[:], in1=xd_b,
                        op=mybir.AluOpType.subtract,
                    )
                    nc.scalar.activation(
                        xs[:], xs[:], mybir.ActivationFunctionType.Square
                    )
                    v = pool.tile([P, C], F32, tag="v")
                    nc.vector.reduce_sum(
                        out=v[:],
                        in_=xs[:].rearrange("p (j c) -> p c j", j=KS),
                        axis=mybir.AxisListType.X,
                    )
                    vt_ps = psum.tile([C, P], F32, space="PSUM", tag="vt")
                    nc.tensor.transpose(out=vt_ps[:], in_=v[:], identity=ident[:])
                    vt = pool.tile([C, P], F32, tag="vts")
                    nc.vector.tensor_copy(out=vt[:], in_=vt_ps[:])
                    h_ps = psum.tile([H, P], F32, space="PSUM", tag="h_ps")
                    nc.tensor.matmul(
                        out=h_ps[:], lhsT=w1k_t[:], rhs=vt[:], start=True, stop=True
                    )
                    nc.scalar.activation(
                        h_g[:, ti * P : (ti + 1) * P],
                        h_ps[:],
                        mybir.ActivationFunctionType.Relu,
                        bias=b1_t[:, :1],
                    )
                for s in range(0, ng * P, 512):
                    e = min(s + 512, ng * P)
                    nc.tensor.matmul(
                        out=o_ps[:, s:e], lhsT=w2_t[:], rhs=h_g[:, s:e],
                        start=True, stop=True,
                    )
                nc.scalar.activation(
                    y_sb[:, : ng * P],
                    o_ps[:, : ng * P],
                    mybir.ActivationFunctionType.Sigmoid,
                    bias=b2_t[:, :1],
                )
                nc.sync.dma_start(
                    out=y[:, g * P : g * P + ng * P], in_=y_sb[:, : ng * P]
                )
    nc.compile()
    return nc


# ------------------------------------------------------------------ driver


def _mlp_consts(W1, b1, W2, b2, k, H, bf16=True):
    import ml_dtypes

    kk = float(np.asarray(k))
    w1k = np.ascontiguousarray(np.asarray(W1, dtype=np.float32) / kk)
    w2v = np.ascontiguousarray(np.asarray(W2, dtype=np.float32).reshape(H, 1))
    if bf16:
        w1k = w1k.astype(ml_dtypes.bfloat16)
        w2v = w2v.astype(ml_dtypes.bfloat16)
    return (
        w1k,
        np.ascontiguousarray(np.asarray(b1, dtype=np.float32).reshape(H, 1)),
        w2v,
        np.ascontiguousarray(np.asarray(b2, dtype=np.float32).reshape(1, 1)),
    )


def _run_indirect(x, src, dst, k, W1, b1, W2, b2):
    from concourse.bass_utils import run_bass_kernel_spmd

    N, C = x.shape
    H = W1.shape[1]
    starts, nbrs, deg = _node_lists(x, np.asarray(src), np.asarray(dst), k)
    K = nbrs.shape[1]
    KS = K + 1
    n_cores = 8
    per_core = max(starts[c + 1] - starts[c] for c in range(n_cores))
    NT = (per_core + P - 1) // P
    n_nodes_pad = NT * P

    key = ("ind", N, C, KS, NT, n_nodes_pad, H)
    if key not in _KERNEL_CACHE:
        _KERNEL_CACHE[key] = _build_indirect(N, C, KS, NT, n_nodes_pad, H)
    nc = _KERNEL_CACHE[key]

    w1k, b1v, w2v, b2v = _mlp_consts(W1, b1, W2, b2, k, H, bf16=False)

    in_maps = []
    for c in range(n_cores):
        lo, hi = starts[c], starts[c + 1]
        idx = np.zeros((n_nodes_pad, KS), dtype=np.int32)
        nb = nbrs[lo:hi]
        own = np.arange(lo, hi, dtype=np.int64)
        nb2 = np.where(nb >= 0, nb, own[:, None])
        idx[: hi - lo, :K] = nb2
        idx[: hi - lo, K] = own
        in_maps.append(
            {"x": x, "idx": idx, "w1k": w1k, "b1": b1v, "w2": w2v, "b2": b2v}
        )

    res = run_bass_kernel_spmd(nc, in_maps, core_ids=list(range(n_cores)))
    out = np.empty((N, 1), dtype=np.float32)
    for c in range(n_cores):
        lo, hi = starts[c], starts[c + 1]
        out[lo:hi, 0] = res.results[c]["y"][0, : hi - lo]
    return out


def _run_gather(x, src, dst, k, W1, b1, W2, b2):
    from concourse.bass_utils import run_bass_kernel_spmd

    N, C = x.shape
    H = W1.shape[1]
    plan = _plan_all(x, np.asarray(src), np.asarray(dst), k)
    n_cores = 8
    n_nodes_pad = plan["n_nodes_pad"]
    starts = plan["starts"]

    key = ("gat", N, C, plan["NT"], n_nodes_pad, H, plan["A"],
           tuple(plan["OVC"].ravel().tolist()))
    if key not in _KERNEL_CACHE:
        _KERNEL_CACHE[key] = _build_gather(plan, H)
    nc = _KERNEL_CACHE[key]

    w1k, b1v, w2v, b2v = _mlp_consts(W1, b1, W2, b2, k, H)
    import ml_dtypes

    iota = np.tile(np.arange(P, dtype=np.float32), (P, 1)).astype(ml_dtypes.bfloat16)
    xq = _build_x_aug(x)

    in_maps = []
    for c in range(n_cores):
        lo, hi = starts[c], starts[c + 1]
        xm2 = np.zeros((n_nodes_pad, C), dtype=np.float16)
        xm2[: hi - lo] = -2.0 * np.asarray(x[lo:hi], dtype=np.float32)
        xu = np.zeros((n_nodes_pad, C), dtype=np.float16)
        xu[: hi - lo] = (
            plan["deg"][lo:hi, None].astype(np.float32)
            * np.square(np.asarray(x[lo:hi], dtype=np.float32))
        )
        m = {
            "idx16": plan["core_data"][c]["idx16"],
            "ovt": plan["core_data"][c]["ovt"],
            "xm2": xm2,
            "xu": xu,
            "iota": iota,
            "w1k": w1k,
            "b1": b1v,
            "w2": w2v,
            "b2": b2v,
        }
        for q in range(NCLASS):
            m[f"xq{q}"] = xq[q]
        in_maps.append(m)

    res = run_bass_kernel_spmd(nc, in_maps, core_ids=list(range(n_cores)))
    out = np.empty((N, 1), dtype=np.float32)
    for c in range(n_cores):
        lo, hi = starts[c], starts[c + 1]
        out[lo:hi, 0] = res.results[c]["y"][0, : hi - lo]
    return out


def kernel(x, src, dst, k, W1, b1, W2, b2):
    global CHUNK
    _install_ntff_hook()
    x = np.ascontiguousarray(np.asarray(x, dtype=np.float32))
    N = x.shape[0]
    chunk = -(-N // NCLASS)  # ceil
    if MODE == "gather" and chunk <= 32767 and x.shape[1] == 128:
        CHUNK = chunk
        return _run_gather(x, src, dst, k, W1, b1, W2, b2)
    return _run_indirect(x, src, dst, k, W1, b1, W2, b2)


def run_traced(**inputs):
    """test.py helper: run with NTFF tracing, return (output, exec_time_ns)."""
    _install_ntff_hook()
    import concourse.bass_utils as bu

    orig = bu.run_bass_kernel_spmd
    holder = {}

    def wrapper(nc, in_maps, core_ids, **kw):
        kw["trace"] = True
        r = orig(nc, in_maps, core_ids, **kw)
        holder["exec_time_ns"] = r.exec_time_ns
        return r

    bu.run_bass_kernel_spmd = wrapper
    try:
        out = kernel(**inputs)
    finally:
        bu.run_bass_kernel_spmd = orig
    return out, holder.get("exec_time_ns")

